# revision 1
# baseline (speedup 1.0000x reference)
"""HCLT probabilistic-circuit kernel for 8 Trainium2 NeuronCores.

Math: the reference collapses algebraically. With
  lp0 + lp1 summed in log space, exp'd, mixed by w_sum, then logsumexp'd,
the whole network is
  out[b] = log( sum_{k,m} w_sum[k] * W0[k,m,x0_b] * W1[k,m,x1_b] )
        = log( A[x0_b, x1_b] ),   A = sum_k w_k * W0[k].T @ W1[k]  (shape [C, C])

Distribution: shard the latent axis k (256) across 8 cores (32 each). Each core
reads only its W shard (134MB/8/2 in bf16 = 8.4MB), computes the partial
A_c = sum_{k in shard} w_k W0[k].T @ W1[k] via PSUM-accumulated matmuls, then
gathers its partial A_c at all 1024 (x0_b, x1_b) positions on-device
(one-hot matmul row-gather + fused mask-dot column-gather). The host sums the
8 partial gathered vectors (the unshard of the k-sharded reduction) and takes
the log.
"""

import sys

import numpy as np

sys.path.insert(0, "/opt/trn_rl_repo")

import ml_dtypes

B, V, M, C = 1024, 2, 256, 256
NCORES = 8
KSH = M // NCORES          # k per core = 32
KM = KSH * M               # flattened contraction rows per core = 8192
NCHUNK = KM // 128         # 64 matmul chunks of 128 rows
NBT = B // 128             # 8 batch tiles

_cache = {}


def _build_program():
    import concourse.bacc as bacc
    import concourse.mybir as mybir
    from concourse.tile import TileContext

    f32 = mybir.dt.float32
    bf16 = mybir.dt.bfloat16

    nc = bacc.Bacc("TRN2", target_bir_lowering=False)

    x0w = nc.dram_tensor("x0w", [128, NCHUNK * C], bf16, kind="ExternalInput")
    x1w = nc.dram_tensor("x1w", [128, NCHUNK * C], bf16, kind="ExternalInput")
    # aux packs f32 [wsc (64) | iota (256) | x1t (8)] per partition
    aux = nc.dram_tensor("aux", [128, NCHUNK + C + NBT], f32, kind="ExternalInput")
    oh0t = nc.dram_tensor("oh0t", [2 * 128, B], bf16, kind="ExternalInput")
    gout = nc.dram_tensor("gout", [128, NBT], f32, kind="ExternalOutput")

    NPIECE = 8
    PW = NCHUNK * C // NPIECE  # 2048 columns per DMA piece

    with TileContext(nc) as tc:
        with (
            tc.tile_pool(name="wp", bufs=1) as wp,
            tc.tile_pool(name="sp", bufs=3) as sp,
            tc.tile_pool(name="rp", bufs=4, space="PSUM") as rp,
            tc.tile_pool(name="apool", bufs=1, space="PSUM") as apool,
        ):
            x0sb = wp.tile([128, NCHUNK * C], bf16, name="x0sb")
            x1sb = wp.tile([128, NCHUNK * C], bf16, name="x1sb")
            x0s = wp.tile([128, NCHUNK * C], bf16, name="x0s")
            auxsb = wp.tile([128, NCHUNK + C + NBT], f32, name="auxsb")
            oh0sb = wp.tile([128, 2 * B], bf16, name="oh0sb")
            oh1sb = wp.tile([128, NBT * C], f32, name="oh1sb")
            gsb = wp.tile([128, NBT], f32, name="gsb")

            nc.sync.dma_start(out=auxsb[:], in_=aux[:])
            wscsb = auxsb[:, 0:NCHUNK]
            iotasb = auxsb[:, NCHUNK : NCHUNK + C]
            x1tsb = auxsb[:, NCHUNK + C : NCHUNK + C + NBT]

            # interleave the W-shard pieces so compute can chase the DMAs
            for p in range(NPIECE):
                sl = slice(p * PW, (p + 1) * PW)
                nc.sync.dma_start(out=x0sb[:, sl], in_=x0w[:, sl])
                nc.sync.dma_start(out=x1sb[:, sl], in_=x1w[:, sl])
            nc.sync.dma_start(out=oh0sb[:, 0:B], in_=oh0t[0:128, :])
            nc.sync.dma_start(out=oh0sb[:, B : 2 * B], in_=oh0t[128:256, :])

            # scale W0 chunks by their (uniform-per-chunk) w_sum factor
            for j in range(NCHUNK):
                sl = slice(j * C, (j + 1) * C)
                nc.vector.tensor_scalar(
                    out=x0s[:, sl],
                    in0=x0sb[:, sl],
                    scalar1=wscsb[:, j : j + 1],
                    scalar2=None,
                    op0=mybir.AluOpType.mult,
                )

            # build the 8 per-batch-tile x1 one-hot masks (needed only at
            # the gather stage; placed after the scales so the first scale
            # op -- which gates the first matmul -- issues as early as
            # possible on the in-order DVE queue)
            for i in range(NBT):
                nc.vector.tensor_scalar(
                    out=oh1sb[:, i * C : (i + 1) * C],
                    in0=iotasb,
                    scalar1=x1tsb[:, i : i + 1],
                    scalar2=None,
                    op0=mybir.AluOpType.is_equal,
                )

            # partial A = sum over 64 chunks of x0s_chunk.T @ x1_chunk
            a_ps = []
            for h in range(2):
                ah = apool.tile([128, C], f32, name=f"a{h}")
                a_ps.append(ah)
            # per DMA piece, run each PSUM half as a contiguous 8-MM
            # burst so LDWEIGHTS overlaps within a same-bank run
            CPP = NCHUNK // NPIECE
            for p in range(NPIECE):
                for h in range(2):
                    for j in range(p * CPP, (p + 1) * CPP):
                        nc.tensor.matmul(
                            a_ps[h],
                            lhsT=x0s[:, j * C + h * 128 : j * C + h * 128 + 128],
                            rhs=x1sb[:, j * C : (j + 1) * C],
                            start=(j == 0),
                            stop=(j == NCHUNK - 1),
                        )

            a_sb = []
            for h in range(2):
                ash = wp.tile([128, C], bf16, name=f"ash{h}")
                nc.vector.tensor_copy(ash, a_ps[h])
                a_sb.append(ash)

            # gather: R[b,:] = A[x0_b,:] via one-hot matmul, then dot with
            # the x1 one-hot row mask (built on-device) and reduce.
            for i in range(NBT):
                r_ps = rp.tile([128, C], mybir.dt.float32, name="r_ps")
                nc.tensor.matmul(
                    r_ps,
                    lhsT=oh0sb[:, i * 128 : (i + 1) * 128],
                    rhs=a_sb[0],
                    start=True,
                    stop=False,
                )
                nc.tensor.matmul(
                    r_ps,
                    lhsT=oh0sb[:, B + i * 128 : B + (i + 1) * 128],
                    rhs=a_sb[1],
                    start=False,
                    stop=True,
                )
                masked = sp.tile([128, C], f32, name="masked")
                nc.vector.tensor_tensor(
                    out=masked,
                    in0=r_ps,
                    in1=oh1sb[:, i * C : (i + 1) * C],
                    op=mybir.AluOpType.mult,
                )
                nc.vector.tensor_reduce(
                    out=gsb[:, i : i + 1],
                    in_=masked,
                    axis=mybir.AxisListType.X,
                    op=mybir.AluOpType.add,
                )

            nc.sync.dma_start(out=gout[:], in_=gsb[:])

    nc.compile()
    return nc


def _prep_inputs(x, W, w_sum):
    bf16 = ml_dtypes.bfloat16
    x = np.asarray(x)
    W = np.asarray(W, dtype=np.float32)
    w_sum = np.asarray(w_sum, dtype=np.float32)

    oh0t = np.zeros((C, B), dtype=bf16)
    oh0t[x[:, 0].astype(np.int64), np.arange(B)] = 1
    iotaf = np.broadcast_to(np.arange(C, dtype=np.float32)[None, :], (128, C))
    x1t = x[:, 1].astype(np.float32).reshape(NBT, 128).T

    in_maps = []
    for c in range(NCORES):
        k0 = c * KSH
        w0 = W[0, k0 : k0 + KSH].reshape(KM, C).astype(bf16)
        w1 = W[1, k0 : k0 + KSH].reshape(KM, C).astype(bf16)
        x0wc = np.ascontiguousarray(
            w0.reshape(NCHUNK, 128, C).transpose(1, 0, 2).reshape(128, NCHUNK * C)
        )
        x1wc = np.ascontiguousarray(
            w1.reshape(NCHUNK, 128, C).transpose(1, 0, 2).reshape(128, NCHUNK * C)
        )
        wsc = np.broadcast_to(
            np.repeat(w_sum[k0 : k0 + KSH], M // 128)[None, :], (128, NCHUNK)
        )
        auxc = np.ascontiguousarray(
            np.concatenate([wsc, iotaf, x1t], axis=1).astype(np.float32)
        )
        in_maps.append({"x0w": x0wc, "x1w": x1wc, "aux": auxc, "oh0t": oh0t})
    return in_maps


def _run(in_maps, **kwargs):
    from concourse.bass_utils import run_bass_kernel_spmd

    if "nc" not in _cache:
        _cache["nc"] = _build_program()
    return run_bass_kernel_spmd(
        _cache["nc"], in_maps, core_ids=list(range(NCORES)), **kwargs
    )


def kernel(x, W, w_sum):
    in_maps = _prep_inputs(x, W, w_sum)
    res = _run(in_maps)
    g = np.zeros((128, NBT), dtype=np.float64)
    for r in res.results:
        g += r["gout"].astype(np.float64)
    vals = g.T.reshape(B)  # b = tile*128 + partition
    return np.log(vals).astype(np.float32)



# revision 2
# speedup vs baseline: 1.3762x; 1.3762x over previous
"""HCLT probabilistic-circuit kernel for 8 Trainium2 NeuronCores.

Math: the reference collapses algebraically. With
  lp0 + lp1 summed in log space, exp'd, mixed by w_sum, then logsumexp'd,
the whole network is
  out[b] = log( sum_{k,m} w_sum[k] * W0[k,m,x0_b] * W1[k,m,x1_b] )
        = log( A[x0_b, x1_b] ),   A = sum_k w_k * W0[k].T @ W1[k]  (shape [C, C])

Distribution: shard the latent axis k (256) across 8 cores (32 each). Each core
reads only its W shard, quantized to fp8e4 (e4m3) with sqrt(w_sum) folded into
both factors plus power-of-two range scales, and computes the partial
A_c = sum_{k in shard} (sqw_k W0[k] S0).T @ (sqw_k W1[k] S1) via PSUM-accumulated
DoubleRow fp8 matmuls (256 contraction rows per instruction at 2x rate).
Each core writes its full partial A_c [256, 256] f32; the host sums the 8
partials, gathers at the 1024 (x0_b, x1_b) positions, removes the scales, and
takes the log. fp8 halves HBM traffic (the bottleneck) vs bf16 and the host
gather removes the on-device one-hot gather stage entirely.
"""

import math
import sys

import numpy as np

sys.path.insert(0, "/opt/trn_rl_repo")

import ml_dtypes

B, V, M, C = 1024, 2, 256, 256
NCORES = 8
KSH = M // NCORES          # k per core = 32
KM = KSH * M               # flattened contraction rows per core = 8192
NC2 = KM // 256            # 32 DoubleRow chunks of 256 rows
NPIECE = 4                 # DMA pieces per W tensor
CPP = NC2 // NPIECE        # chunks per piece

_cache = {}


def _build_program():
    import concourse.bacc as bacc
    import concourse.mybir as mybir
    from concourse.tile import TileContext

    f32 = mybir.dt.float32
    fp8 = mybir.dt.float8e4

    nc = bacc.Bacc("TRN2", target_bir_lowering=False)

    # free-dim layout per partition p:
    #   x0w: [j(NC2), h(2), i(2), m(128)] = P0[r, h*128+m], r = j*256+i*128+p
    #   x1w: [j(NC2), i(2), n(256)]       = P1[r, n]
    x0w = nc.dram_tensor("x0w", [128, NC2 * 512], fp8, kind="ExternalInput")
    x1w = nc.dram_tensor("x1w", [128, NC2 * 512], fp8, kind="ExternalInput")
    # aout free layout: [h(2), n(256)] = A_partial[h*128+p, n]
    aout = nc.dram_tensor("aout", [128, 512], f32, kind="ExternalOutput")

    PW = NC2 * 512 // NPIECE

    with TileContext(nc) as tc:
        with (
            tc.tile_pool(name="wp", bufs=1) as wp,
            tc.tile_pool(name="apool", bufs=1, space="PSUM") as apool,
        ):
            x0sb = wp.tile([128, NC2 * 512], fp8, name="x0sb")
            x1sb = wp.tile([128, NC2 * 512], fp8, name="x1sb")
            asb = wp.tile([128, 512], f32, name="asb")

            # interleave piece DMAs across the two trigger-capable engines so
            # matmuls can chase the transfers
            for p in range(NPIECE):
                sl = slice(p * PW, (p + 1) * PW)
                nc.sync.dma_start(out=x1sb[:, sl], in_=x1w[:, sl])
                nc.scalar.dma_start(out=x0sb[:, sl], in_=x0w[:, sl])

            a_ps = []
            for h in range(2):
                ah = apool.tile([128, C], f32, name=f"a{h}")
                a_ps.append(ah)

            # partial A = sum over 32 DoubleRow chunks (256 rows each)
            for p in range(NPIECE):
                for h in range(2):
                    for j in range(p * CPP, (p + 1) * CPP):
                        lhsT = x0sb[
                            :, (j * 2 + h) * 256 : (j * 2 + h + 1) * 256
                        ].rearrange("p (i m) -> p i m", i=2)
                        rhs = x1sb[:, j * 512 : (j + 1) * 512].rearrange(
                            "p (i n) -> p i n", i=2
                        )
                        nc.tensor.matmul(
                            a_ps[h],
                            lhsT=lhsT,
                            rhs=rhs,
                            start=(j == p * CPP and p == 0),
                            stop=(j == (p + 1) * CPP - 1 and p == NPIECE - 1),
                            perf_mode=mybir.MatmulPerfMode.DoubleRow,
                        )

            for h in range(2):
                nc.vector.tensor_copy(asb[:, h * 256 : (h + 1) * 256], a_ps[h])

            nc.sync.dma_start(out=aout[:], in_=asb[:])

    nc.compile()
    return nc


def _prep_inputs(x, W, w_sum):
    fp8 = ml_dtypes.float8_e4m3
    x = np.asarray(x)
    W = np.asarray(W, dtype=np.float32)
    w_sum = np.asarray(w_sum, dtype=np.float32)

    sq = np.sqrt(w_sum).astype(np.float32)
    P0 = W[0] * sq[:, None, None]  # [M(k), M(m), C]
    P1 = W[1] * sq[:, None, None]
    S0 = 2.0 ** math.floor(math.log2(192.0 / float(P0.max())))
    S1 = 2.0 ** math.floor(math.log2(192.0 / float(P1.max())))
    Q0 = (P0 * np.float32(S0)).astype(fp8)
    Q1 = (P1 * np.float32(S1)).astype(fp8)

    in_maps = []
    for c in range(NCORES):
        k0 = c * KSH
        q0 = Q0[k0 : k0 + KSH].reshape(KM, C)
        q1 = Q1[k0 : k0 + KSH].reshape(KM, C)
        # x0w[p, j, h, i, m] = q0[j*256 + i*128 + p, h*128 + m]
        t0 = q0.reshape(NC2, 2, 128, 2, 128).transpose(2, 0, 3, 1, 4)
        x0wc = np.ascontiguousarray(t0.reshape(128, NC2 * 512))
        # x1w[p, j, i, n] = q1[j*256 + i*128 + p, n]
        t1 = q1.reshape(NC2, 2, 128, C).transpose(2, 0, 1, 3)
        x1wc = np.ascontiguousarray(t1.reshape(128, NC2 * 512))
        in_maps.append({"x0w": x0wc, "x1w": x1wc})
    return in_maps, S0, S1


def _run(in_maps, **kwargs):
    from concourse.bass_utils import run_bass_kernel_spmd

    if "nc" not in _cache:
        _cache["nc"] = _build_program()
    return run_bass_kernel_spmd(
        _cache["nc"], in_maps, core_ids=list(range(NCORES)), **kwargs
    )


def _finish(res, x, S0, S1):
    x = np.asarray(x)
    asum = np.zeros((128, 512), dtype=np.float64)
    for r in res.results:
        asum += r["aout"].astype(np.float64)
    # A[c0, c1] with c0 = h*128 + p
    A = asum.reshape(128, 2, 256).transpose(1, 0, 2).reshape(256, 256)
    vals = A[x[:, 0].astype(np.int64), x[:, 1].astype(np.int64)]
    return (np.log(vals) - math.log(S0 * S1)).astype(np.float32)


def kernel(x, W, w_sum):
    in_maps, S0, S1 = _prep_inputs(x, W, w_sum)
    res = _run(in_maps)
    return _finish(res, x, S0, S1)


# revision 7
# speedup vs baseline: 1.4367x; 1.0440x over previous
"""HCLT probabilistic-circuit kernel for 8 Trainium2 NeuronCores.

Math: the reference collapses algebraically. With
  lp0 + lp1 summed in log space, exp'd, mixed by w_sum, then logsumexp'd,
the whole network is
  out[b] = log( sum_{k,m} w_sum[k] * W0[k,m,x0_b] * W1[k,m,x1_b] )
        = log( A[x0_b, x1_b] ),   A = sum_k w_k * W0[k].T @ W1[k]  (shape [C, C])

Distribution: shard the latent axis k (256) across 8 cores (32 each). Each core
reads its W shard quantized to fp8e4 (sqrt(w_sum) folded into both factors plus
power-of-two range scales) and accumulates the partial A_c via DoubleRow fp8
matmuls (256 contraction rows per instruction at 2x rate). The host sums the 8
partial A_c [256, 256] f32 outputs, gathers at the 1024 (x0_b, x1_b) positions,
removes the scales, and takes the log.

Layout: both W factors live in ONE dram tensor, interleaved per 256-row chunk
(x0-block 512B | x1-block 512B per partition row), so a single DMA trigger
(~0.8us each on the issuing engine) feeds both matmul operands. Pieces are
graduated (tiny first) so the PE starts as soon as possible, and alternate
between the two trigger engines/queues. The partial A is DMA'd straight out
of PSUM, split across both queues.
"""

import math
import sys

import numpy as np

sys.path.insert(0, "/opt/trn_rl_repo")

import ml_dtypes

B, V, M, C = 1024, 2, 256, 256
NCORES = 8
KSH = M // NCORES          # k per core = 32
KM = KSH * M               # flattened contraction rows per core = 8192
NC2 = KM // 256            # 32 DoubleRow chunks of 256 rows
CW = 1024                  # sbuf columns per chunk: [x0 512 | x1 512]
# piece boundaries in chunks: tiny first (early PE start), small last (tail)
PIECES = [0, 1, 3, 6, 10, 14, 18, 22, 26, 30, 32]

_cache = {}


def _build_program():
    import concourse.bacc as bacc
    import concourse.mybir as mybir
    from concourse.tile import TileContext

    bf16 = mybir.dt.bfloat16
    f32 = mybir.dt.float32
    fp8 = mybir.dt.float8e4

    nc = bacc.Bacc("TRN2", target_bir_lowering=False)

    # per-chunk free layout: x0: [h(2), i(2), m(128)] then x1: [i(2), n(256)]
    xw = nc.dram_tensor("xw", [128, NC2 * CW], fp8, kind="ExternalInput")
    warm = nc.dram_tensor("warm", [1, 64], fp8, kind="ExternalInput")
    aout0 = nc.dram_tensor("aout0", [128, C], bf16, kind="ExternalOutput")
    aout1 = nc.dram_tensor("aout1", [128, C], bf16, kind="ExternalOutput")

    with TileContext(nc) as tc:
        with (
            tc.tile_pool(name="wp", bufs=1) as wp,
            tc.tile_pool(name="apool", bufs=1, space="PSUM") as apool,
        ):
            xsb = wp.tile([128, NC2 * CW], fp8, name="xsb")
            warmsb = wp.tile([1, 64], fp8, name="warmsb")

            # tiny first DMA on the scalar queue: pays the queue-init cost
            # before the real transfers need it
            nc.scalar.dma_start(out=warmsb[:], in_=warm[:])
            # graduated pieces alternate between the two trigger engines
            for pi in range(len(PIECES) - 1):
                sl = slice(PIECES[pi] * CW, PIECES[pi + 1] * CW)
                eng = nc.sync if pi % 2 == 0 else nc.scalar
                eng.dma_start(out=xsb[:, sl], in_=xw[:, sl])

            a_ps = []
            for h in range(2):
                ah = apool.tile([128, C], f32, name=f"a{h}")
                a_ps.append(ah)

            for pi in range(len(PIECES) - 1):
                for h in range(2):
                    for j in range(PIECES[pi], PIECES[pi + 1]):
                        lhsT = xsb[
                            :, j * CW + h * 256 : j * CW + (h + 1) * 256
                        ].rearrange("p (i m) -> p i m", i=2)
                        rhs = xsb[:, j * CW + 512 : (j + 1) * CW].rearrange(
                            "p (i n) -> p i n", i=2
                        )
                        nc.tensor.matmul(
                            a_ps[h],
                            lhsT=lhsT,
                            rhs=rhs,
                            start=(j == 0 and pi == 0),
                            stop=(j == NC2 - 1),
                            perf_mode=mybir.MatmulPerfMode.DoubleRow,
                        )

            # PSUM -> SBUF (bf16) on two engines in parallel, then one
            # output DMA per queue
            asb0 = wp.tile([128, C], bf16, name="asb0")
            asb1 = wp.tile([128, C], bf16, name="asb1")
            nc.vector.tensor_copy(asb0, a_ps[0])
            nc.scalar.copy(asb1, a_ps[1])
            nc.sync.dma_start(out=aout0[:], in_=asb0[:])
            nc.scalar.dma_start(out=aout1[:], in_=asb1[:])

    nc.compile()
    return nc


def _prep_inputs(x, W, w_sum):
    fp8 = ml_dtypes.float8_e4m3
    x = np.asarray(x)
    W = np.asarray(W, dtype=np.float32)
    w_sum = np.asarray(w_sum, dtype=np.float32)

    sq = np.sqrt(w_sum).astype(np.float32)
    P0 = W[0] * sq[:, None, None]  # [M(k), M(m), C]
    P1 = W[1] * sq[:, None, None]
    S0 = 2.0 ** math.floor(math.log2(192.0 / float(P0.max())))
    S1 = 2.0 ** math.floor(math.log2(192.0 / float(P1.max())))
    Q0 = (P0 * np.float32(S0)).astype(fp8)
    Q1 = (P1 * np.float32(S1)).astype(fp8)

    warm = np.zeros((1, 64), dtype=fp8)
    in_maps = []
    for c in range(NCORES):
        k0 = c * KSH
        q0 = Q0[k0 : k0 + KSH].reshape(KM, C)
        q1 = Q1[k0 : k0 + KSH].reshape(KM, C)
        # x0 block: [p, j, h, i, m] = q0[j*256 + i*128 + p, h*128 + m]
        t0 = q0.reshape(NC2, 2, 128, 2, 128).transpose(2, 0, 3, 1, 4)
        t0 = t0.reshape(128, NC2, 512)
        # x1 block: [p, j, i, n] = q1[j*256 + i*128 + p, n]
        t1 = q1.reshape(NC2, 2, 128, C).transpose(2, 0, 1, 3)
        t1 = t1.reshape(128, NC2, 512)
        xwc = np.ascontiguousarray(
            np.concatenate([t0, t1], axis=2).reshape(128, NC2 * CW)
        )
        in_maps.append({"xw": xwc, "warm": warm})
    return in_maps, S0, S1


def _run(in_maps, **kwargs):
    from concourse.bass_utils import run_bass_kernel_spmd

    if "nc" not in _cache:
        _cache["nc"] = _build_program()
    return run_bass_kernel_spmd(
        _cache["nc"], in_maps, core_ids=list(range(NCORES)), **kwargs
    )


def _finish(res, x, S0, S1):
    x = np.asarray(x)
    asum = np.zeros((2, 128, C), dtype=np.float64)
    for r in res.results:
        asum[0] += r["aout0"].astype(np.float64)
        asum[1] += r["aout1"].astype(np.float64)
    # A[c0, c1] with c0 = h*128 + p
    A = asum.reshape(256, 256)
    vals = A[x[:, 0].astype(np.int64), x[:, 1].astype(np.int64)]
    return (np.log(vals) - math.log(S0 * S1)).astype(np.float32)


def kernel(x, W, w_sum):
    in_maps, S0, S1 = _prep_inputs(x, W, w_sum)
    res = _run(in_maps)
    return _finish(res, x, S0, S1)


# revision 14
# speedup vs baseline: 1.5843x; 1.1028x over previous
"""HCLT probabilistic-circuit kernel for 8 Trainium2 NeuronCores.

Math: the reference collapses algebraically. With
  lp0 + lp1 summed in log space, exp'd, mixed by w_sum, then logsumexp'd,
the whole network is
  out[b] = log( sum_{k,m} w_sum[k] * W0[k,m,x0_b] * W1[k,m,x1_b] )
        = log( A[x0_b, x1_b] ),   A = sum_k w_k * W0[k].T @ W1[k]  (shape [C, C])

Distribution: shard the latent axis k (256) across 8 cores (32 each). Each core
reads its W shard quantized to fp8e4 (sqrt(w_sum) folded into both factors plus
power-of-two range scales) and accumulates the partial A_c via DoubleRow fp8
matmuls (256 contraction rows per instruction at 2x rate). The host sums the 8
partial A_c [256, 256] f32 outputs, gathers at the 1024 (x0_b, x1_b) positions,
removes the scales, and takes the log.

Layout: both W factors live in ONE dram tensor, interleaved per 256-row chunk
(x0-block 512B | x1-block 512B per partition row), so a single DMA trigger
(~0.8us each on the issuing engine) feeds both matmul operands. Pieces are
graduated (tiny first) so the PE starts as soon as possible, and alternate
between the two trigger engines/queues. The partial A is DMA'd straight out
of PSUM, split across both queues.
"""

import math
import sys

import numpy as np

sys.path.insert(0, "/opt/trn_rl_repo")

import ml_dtypes

B, V, M, C = 1024, 2, 256, 256
NCORES = 8
KSH = M // NCORES          # k per core = 32
KM = KSH * M               # flattened contraction rows per core = 8192
NC2 = KM // 256            # 32 DoubleRow chunks of 256 rows
CW = 1024                  # sbuf columns per chunk: [x0 512 | x1 512]
# pieces as (start_chunk, end_chunk, engine): engine 0 = sync/q1 (ramps fast),
# 1 = scalar/q10 (starts late).  Listed in DMA trigger order per engine;
# MM emission follows expected arrival order (PSUM accumulation is
# order-independent, so chunk order is free).
PIECES = [
    (0, 1, 0),     # q1  c0        ~9.5
    (1, 3, 0),     # q1  c1-2      ~10.6
    (16, 20, 1),   # q10 c16-19    ~11.8
    (3, 6, 0),     # q1  c3-5      ~12.3
    (20, 24, 1),   # q10 c20-23    ~14.0
    (6, 10, 0),    # q1  c6-9      ~14.6
    (24, 28, 1),   # q10 c24-27    ~16.3
    (10, 14, 0),   # q1  c10-13    ~16.9
    (28, 30, 1),   # q10 c28-29    ~17.5
    (14, 16, 0),   # q1  c14-15    ~18.0
    (30, 32, 1),   # q10 c30-31    ~18.6
]

_cache = {}


def _build_program():
    import concourse.bacc as bacc
    import concourse.mybir as mybir
    from concourse.tile import TileContext

    bf16 = mybir.dt.bfloat16
    f32 = mybir.dt.float32
    fp8 = mybir.dt.float8e4

    nc = bacc.Bacc("TRN2", target_bir_lowering=False)

    # per-chunk free layout: x0: [h(2), i(2), m(128)] then x1: [i(2), n(256)]
    xw = nc.dram_tensor("xw", [128, NC2 * CW], fp8, kind="ExternalInput")
    warm = nc.dram_tensor("warm", [128, 512], fp8, kind="ExternalInput")
    aout0 = nc.dram_tensor("aout0", [128, C], bf16, kind="ExternalOutput")
    aout1 = nc.dram_tensor("aout1", [128, C], bf16, kind="ExternalOutput")

    with TileContext(nc) as tc:
        with (
            tc.tile_pool(name="wp", bufs=1) as wp,
            tc.tile_pool(name="apool", bufs=1, space="PSUM") as apool,
        ):
            xsb = wp.tile([128, NC2 * CW], fp8, name="xsb")
            warmsb0 = wp.tile([128, 256], fp8, name="warmsb0")
            warmsb1 = wp.tile([128, 256], fp8, name="warmsb1")

            # tiny first DMAs on both queues: pay queue init and start the
            # DMA-engine clock ramp before the real transfers need it
            nc.sync.dma_start(out=warmsb0[:], in_=warm[:, 0:256])
            nc.scalar.dma_start(out=warmsb1[:], in_=warm[:, 256:512])
            # graduated pieces; each engine's triggers issue in listed order
            for eng_id in (0, 1):
                eng = nc.sync if eng_id == 0 else nc.scalar
                for a, b, e in PIECES:
                    if e == eng_id:
                        sl = slice(a * CW, b * CW)
                        eng.dma_start(out=xsb[:, sl], in_=xw[:, sl])

            a_ps = []
            for h in range(2):
                ah = apool.tile([128, C], f32, name=f"a{h}")
                a_ps.append(ah)

            nmm = [0, 0]
            for a, b, _e in PIECES:
                for h in range(2):
                    for j in range(a, b):
                        lhsT = xsb[
                            :, j * CW + h * 256 : j * CW + (h + 1) * 256
                        ].rearrange("p (i m) -> p i m", i=2)
                        rhs = xsb[:, j * CW + 512 : (j + 1) * CW].rearrange(
                            "p (i n) -> p i n", i=2
                        )
                        nmm[h] += 1
                        nc.tensor.matmul(
                            a_ps[h],
                            lhsT=lhsT,
                            rhs=rhs,
                            start=(nmm[h] == 1),
                            stop=(nmm[h] == NC2),
                            perf_mode=mybir.MatmulPerfMode.DoubleRow,
                        )

            # PSUM -> SBUF (bf16) on two engines in parallel, then one
            # output DMA per queue
            asb0 = wp.tile([128, C], bf16, name="asb0")
            asb1 = wp.tile([128, C], bf16, name="asb1")
            nc.vector.tensor_copy(asb0, a_ps[0])
            nc.scalar.copy(asb1, a_ps[1])
            nc.sync.dma_start(out=aout0[:], in_=asb0[:])
            nc.scalar.dma_start(out=aout1[:], in_=asb1[:])

    nc.compile()
    return nc


def _prep_inputs(x, W, w_sum):
    fp8 = ml_dtypes.float8_e4m3
    x = np.asarray(x)
    W = np.asarray(W, dtype=np.float32)
    w_sum = np.asarray(w_sum, dtype=np.float32)

    sq = np.sqrt(w_sum).astype(np.float32)
    P0 = W[0] * sq[:, None, None]  # [M(k), M(m), C]
    P1 = W[1] * sq[:, None, None]
    S0 = 2.0 ** math.floor(math.log2(192.0 / float(P0.max())))
    S1 = 2.0 ** math.floor(math.log2(192.0 / float(P1.max())))
    Q0 = (P0 * np.float32(S0)).astype(fp8)
    Q1 = (P1 * np.float32(S1)).astype(fp8)

    warm = np.zeros((128, 512), dtype=fp8)
    in_maps = []
    for c in range(NCORES):
        k0 = c * KSH
        q0 = Q0[k0 : k0 + KSH].reshape(KM, C)
        q1 = Q1[k0 : k0 + KSH].reshape(KM, C)
        # x0 block: [p, j, h, i, m] = q0[j*256 + i*128 + p, h*128 + m]
        t0 = q0.reshape(NC2, 2, 128, 2, 128).transpose(2, 0, 3, 1, 4)
        t0 = t0.reshape(128, NC2, 512)
        # x1 block: [p, j, i, n] = q1[j*256 + i*128 + p, n]
        t1 = q1.reshape(NC2, 2, 128, C).transpose(2, 0, 1, 3)
        t1 = t1.reshape(128, NC2, 512)
        xwc = np.ascontiguousarray(
            np.concatenate([t0, t1], axis=2).reshape(128, NC2 * CW)
        )
        in_maps.append({"xw": xwc, "warm": warm})
    return in_maps, S0, S1


def _run(in_maps, **kwargs):
    from concourse.bass_utils import run_bass_kernel_spmd

    if "nc" not in _cache:
        _cache["nc"] = _build_program()
    return run_bass_kernel_spmd(
        _cache["nc"], in_maps, core_ids=list(range(NCORES)), **kwargs
    )


def _finish(res, x, S0, S1):
    x = np.asarray(x)
    asum = np.zeros((2, 128, C), dtype=np.float64)
    for r in res.results:
        asum[0] += r["aout0"].astype(np.float64)
        asum[1] += r["aout1"].astype(np.float64)
    # A[c0, c1] with c0 = h*128 + p
    A = asum.reshape(256, 256)
    vals = A[x[:, 0].astype(np.int64), x[:, 1].astype(np.int64)]
    return (np.log(vals) - math.log(S0 * S1)).astype(np.float32)


def kernel(x, W, w_sum):
    in_maps, S0, S1 = _prep_inputs(x, W, w_sum)
    res = _run(in_maps)
    return _finish(res, x, S0, S1)
